# revision 33
# baseline (speedup 1.0000x reference)
"""Trainium2 Bass kernel for DensityGCNProcessor.

Model: 2-layer GCN over a per-sample kNN graph built from 1-D density values
(K=4 nearest by |density_i - density_j|), symmetric deg^-1/2 normalization on
target indegree, relu after each layer.

Strategy
--------
kNN in a 1-D metric means: after sorting nodes by density, every node's 4
nearest neighbours lie within +/-4 sorted positions, so each aggregation is a
9-diagonal banded matrix in sorted order. The host does all O(N log N) index
math (argsort, band weights with exact reference tie-breaking) and also the
data-dependent *permutation* of the inputs/outputs (gather into sorted order
is part of sharding). The device kernel is then a pure dense pipeline with
zero transposes and zero indirect DMA:

  agg1 (flip):  A1^T[cin,rows] = xs_tile^T @ band1_tile     (TensorE)
  dense1:       H^T[chid,cols] = relu(W1^T @ A1^T + b1)     (TensorE + ACT)
  dense2:       T2[rows,cout]  = (H^T tile)^T @ W2          (TensorE)
  agg2:         out[rows,cout] = relu(band2^T @ T2 + b2)    (TensorE + DVE/ACT)

Node tiles are 120 rows wide so each tile's +/-4-halo window is exactly 128
sorted rows -> every band aggregation is a single K=128 matmul. All operands
are bf16 (tolerance is 2e-2; measured headroom ~30x), all DMAs are contiguous
per partition.

Sharding: 8 cores = 4 batches x 2 rank-halves. Core c handles batch c//2,
sorted ranks [ (c%2)*2048, (c%2)*2048+2048 ).
"""

import numpy as np

# ---------------------------------------------------------------- constants
B = 4
CIN = 256
CHID = 512
COUT = 256
H = W = 64
N = H * W            # 4096 nodes per batch
KNN = 4
BAND = 4             # kNN lies within +/-4 sorted positions
HALF = N // 2        # 2048 ranks per core
TR = 116             # rows per node tile (window = TR + 2*BAND = 124)
WIN = TR + 2 * BAND  # 124
NTA = 18             # node tiles (covers 2088 rows >= 2048 + halo)
NCOL = NTA * TR      # 2088
NCOLP = NCOL + 8     # padded a1T/hT column space (dense2 tile 17 reads 2096)
BCOL = TR + 2 * BAND  # 124 band columns per tile: agg1 reads [0,116), agg2
                      # reads the +4-shifted slice [4,120) + partitions [4,128)

_COMPILED = {}


# ---------------------------------------------------------------- host graph
def _build_band_weights(d_flat):
    """order [N], w9 [N, 9] f32: out_s[r] = sum_o w9[r, o+4] * g_s[r+o]."""
    order = np.argsort(d_flat, kind="stable")
    d_s = d_flat[order]

    offs = np.arange(-BAND, BAND + 1)
    ridx = np.arange(N)[:, None] + offs[None, :]
    valid = (ridx >= 0) & (ridx < N)
    ridx_c = np.clip(ridx, 0, N - 1)
    c = np.abs(d_s[ridx_c] - d_s[:, None]).astype(np.float32)
    c = np.where(valid, c, np.float32(np.inf))
    cand_j = np.where(valid, order[ridx_c], N)

    # reference = stable argsort over the full row: ties by smaller orig index.
    sel = np.lexsort((cand_j, c), axis=1)
    tgt_s = np.take_along_axis(ridx_c, sel[:, 1:KNN + 1], axis=1).reshape(-1)
    src_s = np.repeat(np.arange(N), KNN)

    deg = np.ones(N, dtype=np.float32)
    np.add.at(deg, tgt_s, np.float32(1.0))
    dinv = (np.float32(1.0) / np.sqrt(deg)).astype(np.float32)

    m = np.zeros((N, 9), dtype=np.float32)
    np.add.at(m, (tgt_s, src_s - tgt_s + BAND), np.float32(1.0))
    m[:, BAND] += 1.0  # self loops

    ro = np.arange(N)[:, None] + offs[None, :]
    rov = (ro >= 0) & (ro < N)
    w9 = m * dinv[:, None] * dinv[np.clip(ro, 0, N - 1)] * rov
    return order.astype(np.int64), w9.astype(np.float32)


def _host_graph(density_maps):
    """Per-core index/band tensors. Returns list of 8 dicts."""
    import ml_dtypes
    bf16 = ml_dtypes.bfloat16

    tt = np.arange(NTA)[:, None]
    qq128 = np.arange(128)[None, :]
    rr = np.arange(BCOL)[None, :]
    dd = np.arange(128)[:, None] - np.arange(BCOL)[None, :]    # [128, BCOL]
    okd = (dd >= 0) & (dd <= 2 * BAND)
    dd_c = np.clip(dd, 0, 2 * BAND)

    per_core = []
    for b in range(B):
        d = np.asarray(density_maps[b]).reshape(N).astype(np.float32)
        order, w9g = _build_band_weights(d)
        for half in range(2):
            r0 = half * HALF

            # xs gather: tile t, window row q -> sorted rank r0 - 2*BAND + 116t + q
            rk = r0 - 2 * BAND + TR * tt + qq128                # [NTA, 128]
            ok = (rk >= 0) & (rk < N) & (qq128 < WIN)
            nodes = np.where(ok, order[np.clip(rk, 0, N - 1)], 0)

            # band1 [128 q, NTA t, BCOL r]: A1 row j = 116t + r is rank
            # g1 = r0 - 4 + j; value w9[g1, q - r] for 0 <= q-r <= 8.
            # Columns [116, 124) replicate the next tile's first rows; agg2
            # reads the +4-shifted window (band2[q,u,s] == band1[q+4,u,s+4]).
            g1 = r0 - BAND + TR * tt + rr                       # [NTA, BCOL]
            v1 = (g1 >= 0) & (g1 < N) & (g1 < r0 + HALF + BAND)
            band1 = w9g[np.clip(g1, 0, N - 1)[None], dd_c[:, None, :]]
            band1 *= (v1[None] & okd[:, None, :])

            per_core.append(dict(
                nodes=nodes, ok=ok, order=order,
                band1=np.ascontiguousarray(band1).astype(bf16),
            ))
    return per_core


# ---------------------------------------------------------------- device IR
def build_nc():
    import concourse.bacc as bacc
    import concourse.mybir as mybir
    from concourse.tile import TileContext

    F32 = mybir.dt.float32
    BF16 = mybir.dt.bfloat16
    RELU = mybir.ActivationFunctionType.Relu
    ADD = mybir.AluOpType.add

    nc = bacc.Bacc()
    xs = nc.dram_tensor("xs", [128, NTA, CIN], BF16, kind="ExternalInput")
    band1 = nc.dram_tensor("band1", [128, NTA, BCOL], BF16, kind="ExternalInput")
    w1 = nc.dram_tensor("w1", [128, 2, CHID], BF16, kind="ExternalInput")
    w2 = nc.dram_tensor("w2", [128, 4, COUT], BF16, kind="ExternalInput")
    b1 = nc.dram_tensor("b1", [128, 4], F32, kind="ExternalInput")
    b2rep = nc.dram_tensor("b2rep", [TR, COUT], BF16, kind="ExternalInput")
    out_s = nc.dram_tensor("out_s", [TR, NTA, COUT], BF16, kind="ExternalOutput")

    with TileContext(nc) as tc:
        with (
            tc.tile_pool(name="const", bufs=1) as cpool,
            tc.tile_pool(name="big", bufs=1) as big,
            tc.tile_pool(name="psum", bufs=2, space="PSUM") as pp,
        ):
            # interleave xs (sync HWDGE ring) and band1 (scalar HWDGE ring)
            # per 3-tile group so agg1 can start on tile 0 ASAP
            xs_sb = cpool.tile([128, NTA, CIN], BF16)
            band1_sb = cpool.tile([128, NTA, BCOL], BF16)
            # Each HWDGE ring executes its DMAs serially in FIFO order, so
            # ring assignment = arrival schedule. sync: xs chunks (the agg1
            # critical path). scalar: w1 then band1 chunks in tile order.
            # gpsimd (SWDGE): everything needed later.
            w1_sb = cpool.tile([128, 2, CHID], BF16)
            b1_sb = cpool.tile([128, 4], F32)
            w2_sb = cpool.tile([128, 4, COUT], BF16)
            b2_sb = cpool.tile([TR, COUT], BF16)
            # Few BIG DMAs: small DMAs on a serial HWDGE ring only reach
            # ~100-150 GB/s; ~0.5MB+ transfers run near peak.
            for ch in range(3):
                sl = slice(6 * ch, 6 * (ch + 1))
                nc.sync.dma_start(xs_sb[:, sl, :], xs[:, sl, :])
            nc.scalar.dma_start(band1_sb[:, 0:6, :], band1[:, 0:6, :])
            nc.scalar.dma_start(w1_sb, w1[:, :, :])
            nc.scalar.dma_start(band1_sb[:, 6:NTA, :], band1[:, 6:NTA, :])
            nc.gpsimd.dma_start(b1_sb, b1[:, :])
            nc.gpsimd.dma_start(w2_sb, w2[:, :, :])
            nc.gpsimd.dma_start(b2_sb, b2rep[:, :])

            a1T = big.tile([128, 2, NCOLP], BF16)
            # hT columns are shifted by +4: col 4+j holds a1 col j, cols
            # [0,4) are zero. dense2 tile u then reads cols [116u, 116u+128)
            # giving a 128-row T2 tile whose row p sits at rank r0-8+116u+p,
            # so agg2's band slice is band1[:, u, 4:120] with base 0.
            hT = big.tile([128, 4, NCOLP + 4], BF16)
            t2n = big.tile([128, NTA, COUT], BF16)
            out_sb = big.tile([TR, NTA, COUT], BF16)

            # PE prewarm: ~3.4us of dummy matmuls (one full HAM window) so
            # the clock-gate opens to 2.4 GHz right as the first input chunk
            # lands; they only depend on the DVE memset, so they run under
            # the input-DMA wait.
            warm_sb = cpool.tile([128, 512], BF16)
            nc.vector.memset(warm_sb, 0.25)
            for _ in range(8):
                psW = pp.tile([128, 512], F32, tag="d1", bufs=3)
                nc.tensor.matmul(psW, lhsT=warm_sb[:, 0:128], rhs=warm_sb,
                                 start=True, stop=True)

            # zero the padded tail/head so dense1/dense2 read finite values
            # (on DVE: the gpsimd queue is busy generating SWDGE descriptors)
            for cb in range(2):
                nc.vector.memset(a1T[:, cb, NCOL:NCOLP], 0.0)
            for mb in range(4):
                nc.vector.memset(hT[:, mb, 0:4], 0.0)

            MAX = mybir.AluOpType.max
            COPY = mybir.ActivationFunctionType.Copy

            def emit_agg1(t):
                # A1^T tile = xs_tile^T @ band1_tile (K = 124-row window)
                for cb in range(2):
                    psA = pp.tile([128, TR], F32, tag="sm")
                    nc.tensor.matmul(psA,
                                     lhsT=xs_sb[0:WIN, t, 128 * cb:128 * (cb + 1)],
                                     rhs=band1_sb[0:WIN, t, 0:TR],
                                     start=True, stop=True)
                    dst = a1T[:, cb, TR * t:TR * (t + 1)]
                    if cb == 0:
                        nc.vector.tensor_copy(dst, psA)
                    else:
                        nc.scalar.activation(dst, psA, COPY)

            def emit_d1(lo, hi, mb):
                # H^T block = relu(W1^T @ A1^T + b1)
                psH = pp.tile([128, 512], F32, tag="d1", bufs=3)
                for kb in range(2):
                    nc.tensor.matmul(psH[:, 0:hi - lo],
                                     lhsT=w1_sb[:, kb, 128 * mb:128 * (mb + 1)],
                                     rhs=a1T[:, kb, lo:hi],
                                     start=(kb == 0), stop=(kb == 1))
                if mb % 2 == 0:
                    nc.scalar.activation(hT[:, mb, lo + 4:hi + 4],
                                         psH[:, 0:hi - lo], RELU,
                                         bias=b1_sb[:, mb:mb + 1], scale=1.0)
                else:
                    nc.vector.tensor_scalar(
                        out=hT[:, mb, lo + 4:hi + 4], in0=psH[:, 0:hi - lo],
                        scalar1=b1_sb[:, mb:mb + 1], scalar2=0.0,
                        op0=ADD, op1=MAX)

            def emit_d2(u):
                psT = pp.tile([128, COUT], F32, tag="d2")
                for kb in range(4):
                    nc.tensor.matmul(psT,
                                     lhsT=hT[:, kb, TR * u:TR * u + 128],
                                     rhs=w2_sb[:, kb, :],
                                     start=(kb == 0), stop=(kb == 3))
                if u % 2 == 0:
                    nc.scalar.activation(t2n[:, u, :], psT, COPY)
                else:
                    nc.vector.tensor_copy(t2n[:, u, :], psT)

            def emit_a2(u, flush=None):
                # band2[q,u,s] == band1[q+4,u,s+4]; with hT's +4 column
                # shift, t2n row p sits at rank r0-8+116u+p, so the band
                # slice is a pure free-dim shift with partition base 0.
                psO = pp.tile([TR, COUT], F32, tag="sm")
                nc.tensor.matmul(psO,
                                 lhsT=band1_sb[:, u, 4:TR + 4],
                                 rhs=t2n[:, u, :], start=True, stop=True)
                nc.vector.tensor_tensor(out=out_sb[:, u, :], in0=psO,
                                        in1=b2_sb, op=ADD)
                nc.scalar.activation(out_sb[:, u, :], out_sb[:, u, :], RELU)
                if flush is not None:
                    lo, hi = flush
                    nc.sync.dma_start(out_s[:, lo:hi, :],
                                      out_sb[:, lo:hi, :])

            # ---------------- fused pipeline, chasing the input DMA stream:
            # per 6-tile chunk: agg1 -> dense1 (348-col blocks) -> all dense2/
            # agg2 tiles whose inputs just completed. PE work per chunk
            # (~6.5us) far exceeds the chunk arrival cadence (~2us), so the
            # PE never stalls (or re-throttles) once chunk 0 lands.
            # dense2/agg2 order: tile 17 (the 8-valid-row partial tile) is
            # finished early in the last chunk so the final flush is tiny.
            d2_order = list(range(11)) + [11, 12, 13, 17, 14, 15, 16]
            flushes = {2: (0, 3), 5: (3, 6), 8: (6, 9), 11: (9, 12),
                       13: (12, 14), 17: (17, 18), 15: (14, 16), 16: (16, 17)}
            CW = 6 * TR   # a1/h columns per 6-tile chunk
            pos = 0       # index into d2_order for d2; a2 trails by one
            for ch in range(3):
                for t in range(6 * ch, 6 * ch + 6):
                    emit_agg1(t)
                for half_blk in range(2):
                    lo = CW * ch + 348 * half_blk
                    hi = lo + 348
                    if ch == 2 and half_blk == 1:
                        hi = NCOLP
                    for mb in range(4):
                        emit_d1(lo, hi, mb)
                # dense2 tile u reads hT cols [116u, 116u+128)
                u_max = NTA if ch == 2 else (CW * (ch + 1) - BCOL) // TR + 1
                while pos < len(d2_order) and (ch == 2 or d2_order[pos] < u_max):
                    emit_d2(d2_order[pos])
                    pos += 1
                    if pos >= 2:
                        u = d2_order[pos - 2]
                        emit_a2(u, flushes.get(u))
            u = d2_order[-1]
            emit_a2(u, flushes.get(u))

    nc.compile()
    return nc


# ---------------------------------------------------------------- host glue
def make_in_maps(density_maps, feature_maps, W1, b1, W2, b2):
    import ml_dtypes
    bf16 = ml_dtypes.bfloat16

    graph = _host_graph(density_maps)
    fm = np.asarray(feature_maps, dtype=np.float32)
    fmT = [np.ascontiguousarray(fm[b].reshape(CIN, N).T) for b in range(B)]

    w1d = np.ascontiguousarray(
        np.asarray(W1, np.float32).reshape(2, 128, CHID).transpose(1, 0, 2)
    ).astype(bf16)
    w2d = np.ascontiguousarray(
        np.asarray(W2, np.float32).reshape(4, 128, COUT).transpose(1, 0, 2)
    ).astype(bf16)
    b1d = np.ascontiguousarray(np.asarray(b1, np.float32).reshape(4, 128).T)
    b2d = np.ascontiguousarray(
        np.broadcast_to(np.asarray(b2, np.float32), (TR, COUT))).astype(bf16)

    in_maps = []
    for c in range(8):
        g = graph[c]
        xs = fmT[c // 2][g["nodes"].reshape(-1)].reshape(NTA, 128, CIN)
        xs[~g["ok"]] = 0.0
        xs_dev = xs.transpose(1, 0, 2).astype(bf16)
        in_maps.append({
            "xs": xs_dev, "band1": g["band1"],
            "w1": w1d, "w2": w2d, "b1": b1d, "b2rep": b2d,
        })
    return in_maps, graph


def kernel(density_maps, feature_maps, W1, b1, W2, b2):
    from concourse.bass_utils import run_bass_kernel_spmd

    if "nc" not in _COMPILED:
        _COMPILED["nc"] = build_nc()
    nc = _COMPILED["nc"]

    in_maps, graph = make_in_maps(density_maps, feature_maps, W1, b1, W2, b2)
    res = run_bass_kernel_spmd(nc, in_maps, core_ids=list(range(8)))

    out = np.empty((B, N, COUT), dtype=np.float32)
    for b in range(B):
        order = graph[2 * b]["order"]
        for half in range(2):
            o = np.asarray(res.results[2 * b + half]["out_s"]).astype(np.float32)
            vals = o.transpose(1, 0, 2).reshape(NCOL, COUT)[:HALF]
            out[b][order[half * HALF:(half + 1) * HALF]] = vals
    return np.ascontiguousarray(
        out.reshape(B, H, W, COUT).transpose(0, 3, 1, 2)).astype(np.float32)


# revision 37
# speedup vs baseline: 1.0557x; 1.0557x over previous
"""Trainium2 Bass kernel for DensityGCNProcessor.

Model: 2-layer GCN over a per-sample kNN graph built from 1-D density values
(K=4 nearest by |density_i - density_j|), symmetric deg^-1/2 normalization on
target indegree, relu after each layer.

Strategy
--------
kNN in a 1-D metric means: after sorting nodes by density, every node's 4
nearest neighbours lie within +/-4 sorted positions, so each aggregation is a
9-diagonal banded matrix in sorted order. The host does all O(N log N) index
math (argsort, band weights with exact reference tie-breaking) and also the
data-dependent *permutation* of the inputs/outputs (gather into sorted order
is part of sharding). The device kernel is then a pure dense pipeline with
zero transposes and zero indirect DMA:

  agg1 (flip):  A1^T[cin,rows] = xs_tile^T @ band1_tile     (TensorE)
  dense1:       H^T[chid,cols] = relu(W1^T @ A1^T + b1)     (TensorE + ACT)
  dense2:       T2[rows,cout]  = (H^T tile)^T @ W2          (TensorE)
  agg2:         out[rows,cout] = relu(band2^T @ T2 + b2)    (TensorE + DVE/ACT)

Node tiles are 120 rows wide so each tile's +/-4-halo window is exactly 128
sorted rows -> every band aggregation is a single K=128 matmul. All operands
are bf16 (tolerance is 2e-2; measured headroom ~30x), all DMAs are contiguous
per partition.

Sharding: 8 cores = 4 batches x 2 rank-halves. Core c handles batch c//2,
sorted ranks [ (c%2)*2048, (c%2)*2048+2048 ).
"""

import numpy as np

# ---------------------------------------------------------------- constants
B = 4
CIN = 256
CHID = 512
COUT = 256
H = W = 64
N = H * W            # 4096 nodes per batch
KNN = 4
BAND = 4             # kNN lies within +/-4 sorted positions
HALF = N // 2        # 2048 ranks per core
TR = 116             # rows per node tile (window = TR + 2*BAND = 124)
WIN = TR + 2 * BAND  # 124
NTA = 18             # node tiles (covers 2088 rows >= 2048 + halo)
NCOL = NTA * TR      # 2088
NCOLP = NCOL + 8     # padded a1T/hT column space (dense2 tile 17 reads 2096)
BCOL = TR + 2 * BAND  # 124 band columns per tile: agg1 reads [0,116), agg2
                      # reads the +4-shifted slice [4,120) + partitions [4,128)

_COMPILED = {}


# ---------------------------------------------------------------- host graph
def _build_band_weights(d_flat):
    """order [N], w9 [N, 9] f32: out_s[r] = sum_o w9[r, o+4] * g_s[r+o]."""
    order = np.argsort(d_flat, kind="stable")
    d_s = d_flat[order]

    offs = np.arange(-BAND, BAND + 1)
    ridx = np.arange(N)[:, None] + offs[None, :]
    valid = (ridx >= 0) & (ridx < N)
    ridx_c = np.clip(ridx, 0, N - 1)
    c = np.abs(d_s[ridx_c] - d_s[:, None]).astype(np.float32)
    c = np.where(valid, c, np.float32(np.inf))
    cand_j = np.where(valid, order[ridx_c], N)

    # reference = stable argsort over the full row: ties by smaller orig index.
    sel = np.lexsort((cand_j, c), axis=1)
    tgt_s = np.take_along_axis(ridx_c, sel[:, 1:KNN + 1], axis=1).reshape(-1)
    src_s = np.repeat(np.arange(N), KNN)

    deg = np.ones(N, dtype=np.float32)
    np.add.at(deg, tgt_s, np.float32(1.0))
    dinv = (np.float32(1.0) / np.sqrt(deg)).astype(np.float32)

    m = np.zeros((N, 9), dtype=np.float32)
    np.add.at(m, (tgt_s, src_s - tgt_s + BAND), np.float32(1.0))
    m[:, BAND] += 1.0  # self loops

    ro = np.arange(N)[:, None] + offs[None, :]
    rov = (ro >= 0) & (ro < N)
    w9 = m * dinv[:, None] * dinv[np.clip(ro, 0, N - 1)] * rov
    return order.astype(np.int64), w9.astype(np.float32)


def _host_graph(density_maps):
    """Per-core index/band tensors. Returns list of 8 dicts."""
    import ml_dtypes
    bf16 = ml_dtypes.bfloat16

    tt = np.arange(NTA)[:, None]
    qq128 = np.arange(128)[None, :]
    rr = np.arange(BCOL)[None, :]
    dd = np.arange(128)[:, None] - np.arange(BCOL)[None, :]    # [128, BCOL]
    okd = (dd >= 0) & (dd <= 2 * BAND)
    dd_c = np.clip(dd, 0, 2 * BAND)

    per_core = []
    for b in range(B):
        d = np.asarray(density_maps[b]).reshape(N).astype(np.float32)
        order, w9g = _build_band_weights(d)
        for half in range(2):
            r0 = half * HALF

            # xs gather: tile t, window row q -> sorted rank r0 - 2*BAND + 116t + q
            rk = r0 - 2 * BAND + TR * tt + qq128                # [NTA, 128]
            ok = (rk >= 0) & (rk < N) & (qq128 < WIN)
            nodes = np.where(ok, order[np.clip(rk, 0, N - 1)], 0)

            # band1 [128 q, NTA t, BCOL r]: A1 row j = 116t + r is rank
            # g1 = r0 - 4 + j; value w9[g1, q - r] for 0 <= q-r <= 8.
            # Columns [116, 124) replicate the next tile's first rows; agg2
            # reads the +4-shifted window (band2[q,u,s] == band1[q+4,u,s+4]).
            g1 = r0 - BAND + TR * tt + rr                       # [NTA, BCOL]
            v1 = (g1 >= 0) & (g1 < N) & (g1 < r0 + HALF + BAND)
            band1 = w9g[np.clip(g1, 0, N - 1)[None], dd_c[:, None, :]]
            band1 *= (v1[None] & okd[:, None, :])

            per_core.append(dict(
                nodes=nodes, ok=ok, order=order,
                band1=np.ascontiguousarray(band1).astype(bf16),
            ))
    return per_core


# ---------------------------------------------------------------- device IR
def build_nc():
    import concourse.bacc as bacc
    import concourse.mybir as mybir
    from concourse.tile import TileContext

    F32 = mybir.dt.float32
    BF16 = mybir.dt.bfloat16
    RELU = mybir.ActivationFunctionType.Relu
    ADD = mybir.AluOpType.add

    nc = bacc.Bacc()
    xs = nc.dram_tensor("xs", [128, NTA, CIN], BF16, kind="ExternalInput")
    band1 = nc.dram_tensor("band1", [128, NTA, BCOL], BF16, kind="ExternalInput")
    w1 = nc.dram_tensor("w1", [128, 2, CHID], BF16, kind="ExternalInput")
    w2 = nc.dram_tensor("w2", [128, 4, COUT], BF16, kind="ExternalInput")
    b1 = nc.dram_tensor("b1", [128, 4], F32, kind="ExternalInput")
    b2rep = nc.dram_tensor("b2rep", [TR, COUT], BF16, kind="ExternalInput")
    out_s = nc.dram_tensor("out_s", [TR, NTA, COUT], BF16, kind="ExternalOutput")

    with TileContext(nc) as tc:
        with (
            tc.tile_pool(name="const", bufs=1) as cpool,
            tc.tile_pool(name="big", bufs=1) as big,
            tc.tile_pool(name="psum", bufs=2, space="PSUM") as pp,
        ):
            # interleave xs (sync HWDGE ring) and band1 (scalar HWDGE ring)
            # per 3-tile group so agg1 can start on tile 0 ASAP
            xs_sb = cpool.tile([128, NTA, CIN], BF16)
            band1_sb = cpool.tile([128, NTA, BCOL], BF16)
            # Each HWDGE ring executes its DMAs serially in FIFO order, so
            # ring assignment = arrival schedule. sync: xs chunks (the agg1
            # critical path). scalar: w1 then band1 chunks in tile order.
            # gpsimd (SWDGE): everything needed later.
            w1_sb = cpool.tile([128, 2, CHID], BF16)
            b1_sb = cpool.tile([128, 4], F32)
            w2_sb = cpool.tile([128, 4, COUT], BF16)
            b2_sb = cpool.tile([TR, COUT], BF16)
            # Few BIG DMAs: small DMAs on a serial HWDGE ring only reach
            # ~100-150 GB/s; ~0.5MB+ transfers run near peak.
            for ch in range(3):
                sl = slice(6 * ch, 6 * (ch + 1))
                nc.sync.dma_start(xs_sb[:, sl, :], xs[:, sl, :])
            nc.scalar.dma_start(band1_sb[:, 0:6, :], band1[:, 0:6, :])
            nc.scalar.dma_start(w1_sb, w1[:, :, :])
            nc.scalar.dma_start(band1_sb[:, 6:NTA, :], band1[:, 6:NTA, :])
            nc.gpsimd.dma_start(b1_sb, b1[:, :])
            nc.gpsimd.dma_start(w2_sb, w2[:, :, :])
            nc.gpsimd.dma_start(b2_sb, b2rep[:, :])

            a1T = big.tile([128, 2, NCOLP], BF16)
            # hT columns are shifted by +4: col 4+j holds a1 col j, cols
            # [0,4) are zero. dense2 tile u then reads cols [116u, 116u+128)
            # giving a 128-row T2 tile whose row p sits at rank r0-8+116u+p,
            # so agg2's band slice is band1[:, u, 4:120] with base 0.
            hT = big.tile([128, 4, NCOLP + 4], BF16)
            t2n = big.tile([128, NTA, COUT], BF16)
            out_sb = big.tile([TR, NTA, COUT], BF16)

            # PE prewarm: ~3.4us of dummy matmuls (one full HAM window) so
            # the clock-gate opens to 2.4 GHz right as the first input chunk
            # lands; they only depend on the DVE memset, so they run under
            # the input-DMA wait.
            warm_sb = cpool.tile([128, 512], BF16)
            nc.vector.memset(warm_sb, 0.25)
            for _ in range(8):
                psW = pp.tile([128, 512], F32, tag="d1", bufs=3)
                nc.tensor.matmul(psW, lhsT=warm_sb[:, 0:128], rhs=warm_sb,
                                 start=True, stop=True)

            # zero the padded tail/head so dense1/dense2 read finite values
            # (on DVE: the gpsimd queue is busy generating SWDGE descriptors)
            for cb in range(2):
                nc.vector.memset(a1T[:, cb, NCOL:NCOLP], 0.0)
            for mb in range(4):
                nc.vector.memset(hT[:, mb, 0:4], 0.0)

            MAX = mybir.AluOpType.max
            COPY = mybir.ActivationFunctionType.Copy

            def emit_agg1(t):
                # A1^T tile = xs_tile^T @ band1_tile (K = 124-row window).
                # Both cin-halves land in one 2-region psum tile -> a single
                # drain, so the PE issues back-to-back (keeps HAM warm).
                psA = pp.tile([128, 2, TR], F32, tag="sm", bufs=3)
                for cb in range(2):
                    nc.tensor.matmul(psA[:, cb, :],
                                     lhsT=xs_sb[0:WIN, t, 128 * cb:128 * (cb + 1)],
                                     rhs=band1_sb[0:WIN, t, 0:TR],
                                     start=True, stop=True)
                dst = a1T[:, :, TR * t:TR * (t + 1)]
                if t % 2 == 0:
                    nc.vector.tensor_copy(dst, psA)
                else:
                    nc.scalar.activation(dst, psA, COPY)

            def emit_d1(lo, hi, mb):
                # H^T block = relu(W1^T @ A1^T + b1)
                psH = pp.tile([128, 512], F32, tag="d1", bufs=3)
                for kb in range(2):
                    nc.tensor.matmul(psH[:, 0:hi - lo],
                                     lhsT=w1_sb[:, kb, 128 * mb:128 * (mb + 1)],
                                     rhs=a1T[:, kb, lo:hi],
                                     start=(kb == 0), stop=(kb == 1))
                if mb % 2 == 0:
                    nc.scalar.activation(hT[:, mb, lo + 4:hi + 4],
                                         psH[:, 0:hi - lo], RELU,
                                         bias=b1_sb[:, mb:mb + 1], scale=1.0)
                else:
                    nc.vector.tensor_scalar(
                        out=hT[:, mb, lo + 4:hi + 4], in0=psH[:, 0:hi - lo],
                        scalar1=b1_sb[:, mb:mb + 1], scalar2=0.0,
                        op0=ADD, op1=MAX)

            def emit_d2(us):
                # one psum tile + one drain per GROUP of consecutive tiles
                n = len(us)
                psT = pp.tile([128, 2, COUT], F32, tag="d2")
                for i, u in enumerate(us):
                    for kb in range(4):
                        nc.tensor.matmul(psT[:, i, :],
                                         lhsT=hT[:, kb, TR * u:TR * u + 128],
                                         rhs=w2_sb[:, kb, :],
                                         start=(kb == 0), stop=(kb == 3))
                dst = t2n[:, us[0]:us[0] + n, :]
                if us[0] % 2 == 0:
                    nc.scalar.activation(dst, psT[:, 0:n, :], COPY)
                else:
                    nc.vector.tensor_copy(dst, psT[:, 0:n, :])

            def emit_a2(u, flush=None):
                # band2[q,u,s] == band1[q+4,u,s+4]; with hT's +4 column
                # shift, t2n row p sits at rank r0-8+116u+p, so the band
                # slice is a pure free-dim shift with partition base 0.
                psO = pp.tile([TR, COUT], F32, tag="sm", bufs=3)
                nc.tensor.matmul(psO,
                                 lhsT=band1_sb[:, u, 4:TR + 4],
                                 rhs=t2n[:, u, :], start=True, stop=True)
                nc.vector.tensor_tensor(out=out_sb[:, u, :], in0=psO,
                                        in1=b2_sb, op=ADD)
                nc.scalar.activation(out_sb[:, u, :], out_sb[:, u, :], RELU)
                if flush is not None:
                    lo, hi = flush
                    nc.sync.dma_start(out_s[:, lo:hi, :],
                                      out_sb[:, lo:hi, :])

            # ---------------- fused pipeline, chasing the input DMA stream:
            # per 6-tile chunk: agg1 -> dense1 (348-col blocks) -> all dense2/
            # agg2 tiles whose inputs just completed. PE work per chunk
            # (~6.5us) far exceeds the chunk arrival cadence (~2us), so the
            # PE never stalls (or re-throttles) once chunk 0 lands.
            # dense2/agg2 in groups; tile 17 (the 8-valid-row partial tile)
            # is finished early in the last chunk so the final flush is tiny.
            d2_groups = [[(0, 1), (2, 3), (4,)],
                         [(5, 6), (7, 8), (9, 10)],
                         [(11, 12), (13,), (17,), (14, 15), (16,)]]
            flushes = {2: (0, 3), 5: (3, 6), 8: (6, 9), 11: (9, 12),
                       13: (12, 14), 17: (17, 18), 15: (14, 16), 16: (16, 17)}
            CW = 6 * TR   # a1/h columns per 6-tile chunk
            pend = None   # group whose agg2 is still outstanding
            for ch in range(3):
                for t in range(6 * ch, 6 * ch + 6):
                    emit_agg1(t)
                for half_blk in range(2):
                    lo = CW * ch + 348 * half_blk
                    hi = lo + 348
                    if ch == 2 and half_blk == 1:
                        hi = NCOLP
                    for mb in range(4):
                        emit_d1(lo, hi, mb)
                for grp in d2_groups[ch]:
                    emit_d2(list(grp))
                    if pend is not None:
                        for u in pend:
                            emit_a2(u, flushes.get(u))
                    pend = grp
            for u in pend:
                emit_a2(u, flushes.get(u))

    nc.compile()
    return nc


# ---------------------------------------------------------------- host glue
def make_in_maps(density_maps, feature_maps, W1, b1, W2, b2):
    import ml_dtypes
    bf16 = ml_dtypes.bfloat16

    graph = _host_graph(density_maps)
    fm = np.asarray(feature_maps, dtype=np.float32)
    fmT = [np.ascontiguousarray(fm[b].reshape(CIN, N).T) for b in range(B)]

    w1d = np.ascontiguousarray(
        np.asarray(W1, np.float32).reshape(2, 128, CHID).transpose(1, 0, 2)
    ).astype(bf16)
    w2d = np.ascontiguousarray(
        np.asarray(W2, np.float32).reshape(4, 128, COUT).transpose(1, 0, 2)
    ).astype(bf16)
    b1d = np.ascontiguousarray(np.asarray(b1, np.float32).reshape(4, 128).T)
    b2d = np.ascontiguousarray(
        np.broadcast_to(np.asarray(b2, np.float32), (TR, COUT))).astype(bf16)

    in_maps = []
    for c in range(8):
        g = graph[c]
        xs = fmT[c // 2][g["nodes"].reshape(-1)].reshape(NTA, 128, CIN)
        xs[~g["ok"]] = 0.0
        xs_dev = xs.transpose(1, 0, 2).astype(bf16)
        in_maps.append({
            "xs": xs_dev, "band1": g["band1"],
            "w1": w1d, "w2": w2d, "b1": b1d, "b2rep": b2d,
        })
    return in_maps, graph


def kernel(density_maps, feature_maps, W1, b1, W2, b2):
    from concourse.bass_utils import run_bass_kernel_spmd

    if "nc" not in _COMPILED:
        _COMPILED["nc"] = build_nc()
    nc = _COMPILED["nc"]

    in_maps, graph = make_in_maps(density_maps, feature_maps, W1, b1, W2, b2)
    res = run_bass_kernel_spmd(nc, in_maps, core_ids=list(range(8)))

    out = np.empty((B, N, COUT), dtype=np.float32)
    for b in range(B):
        order = graph[2 * b]["order"]
        for half in range(2):
            o = np.asarray(res.results[2 * b + half]["out_s"]).astype(np.float32)
            vals = o.transpose(1, 0, 2).reshape(NCOL, COUT)[:HALF]
            out[b][order[half * HALF:(half + 1) * HALF]] = vals
    return np.ascontiguousarray(
        out.reshape(B, H, W, COUT).transpose(0, 3, 1, 2)).astype(np.float32)


# revision 39
# speedup vs baseline: 1.0630x; 1.0069x over previous
"""Trainium2 Bass kernel for DensityGCNProcessor.

Model: 2-layer GCN over a per-sample kNN graph built from 1-D density values
(K=4 nearest by |density_i - density_j|), symmetric deg^-1/2 normalization on
target indegree, relu after each layer.

Strategy
--------
kNN in a 1-D metric means: after sorting nodes by density, every node's 4
nearest neighbours lie within +/-4 sorted positions, so each aggregation is a
9-diagonal banded matrix in sorted order. The host does all O(N log N) index
math (argsort, band weights with exact reference tie-breaking) and also the
data-dependent *permutation* of the inputs/outputs (gather into sorted order
is part of sharding). The device kernel is then a pure dense pipeline with
zero transposes and zero indirect DMA:

  agg1 (flip):  A1^T[cin,rows] = xs_tile^T @ band1_tile     (TensorE)
  dense1:       H^T[chid,cols] = relu(W1^T @ A1^T + b1)     (TensorE + ACT)
  dense2:       T2[rows,cout]  = (H^T tile)^T @ W2          (TensorE)
  agg2:         out[rows,cout] = relu(band2^T @ T2 + b2)    (TensorE + DVE/ACT)

Node tiles are 120 rows wide so each tile's +/-4-halo window is exactly 128
sorted rows -> every band aggregation is a single K=128 matmul. All operands
are bf16 (tolerance is 2e-2; measured headroom ~30x), all DMAs are contiguous
per partition.

Sharding: 8 cores = 4 batches x 2 rank-halves. Core c handles batch c//2,
sorted ranks [ (c%2)*2048, (c%2)*2048+2048 ).
"""

import numpy as np

# ---------------------------------------------------------------- constants
B = 4
CIN = 256
CHID = 512
COUT = 256
H = W = 64
N = H * W            # 4096 nodes per batch
KNN = 4
BAND = 4             # kNN lies within +/-4 sorted positions
HALF = N // 2        # 2048 ranks per core
TR = 116             # rows per node tile (window = TR + 2*BAND = 124)
WIN = TR + 2 * BAND  # 124
NTA = 18             # node tiles (covers 2088 rows >= 2048 + halo)
NCOL = NTA * TR      # 2088
NCOLP = NCOL + 8     # padded a1T/hT column space (dense2 tile 17 reads 2096)
BCOL = TR + 2 * BAND  # 124 band columns per tile: agg1 reads [0,116), agg2
                      # reads the +4-shifted slice [4,120) + partitions [4,128)

_COMPILED = {}


# ---------------------------------------------------------------- host graph
def _build_band_weights(d_flat):
    """order [N], w9 [N, 9] f32: out_s[r] = sum_o w9[r, o+4] * g_s[r+o]."""
    order = np.argsort(d_flat, kind="stable")
    d_s = d_flat[order]

    offs = np.arange(-BAND, BAND + 1)
    ridx = np.arange(N)[:, None] + offs[None, :]
    valid = (ridx >= 0) & (ridx < N)
    ridx_c = np.clip(ridx, 0, N - 1)
    c = np.abs(d_s[ridx_c] - d_s[:, None]).astype(np.float32)
    c = np.where(valid, c, np.float32(np.inf))
    cand_j = np.where(valid, order[ridx_c], N)

    # reference = stable argsort over the full row: ties by smaller orig index.
    sel = np.lexsort((cand_j, c), axis=1)
    tgt_s = np.take_along_axis(ridx_c, sel[:, 1:KNN + 1], axis=1).reshape(-1)
    src_s = np.repeat(np.arange(N), KNN)

    deg = np.ones(N, dtype=np.float32)
    np.add.at(deg, tgt_s, np.float32(1.0))
    dinv = (np.float32(1.0) / np.sqrt(deg)).astype(np.float32)

    m = np.zeros((N, 9), dtype=np.float32)
    np.add.at(m, (tgt_s, src_s - tgt_s + BAND), np.float32(1.0))
    m[:, BAND] += 1.0  # self loops

    ro = np.arange(N)[:, None] + offs[None, :]
    rov = (ro >= 0) & (ro < N)
    w9 = m * dinv[:, None] * dinv[np.clip(ro, 0, N - 1)] * rov
    return order.astype(np.int64), w9.astype(np.float32)


def _host_graph(density_maps):
    """Per-core index/band tensors. Returns list of 8 dicts."""
    import ml_dtypes
    bf16 = ml_dtypes.bfloat16

    tt = np.arange(NTA)[:, None]
    qq128 = np.arange(128)[None, :]
    rr = np.arange(BCOL)[None, :]
    dd = np.arange(128)[:, None] - np.arange(BCOL)[None, :]    # [128, BCOL]
    okd = (dd >= 0) & (dd <= 2 * BAND)
    dd_c = np.clip(dd, 0, 2 * BAND)

    per_core = []
    for b in range(B):
        d = np.asarray(density_maps[b]).reshape(N).astype(np.float32)
        order, w9g = _build_band_weights(d)
        for half in range(2):
            r0 = half * HALF

            # xs gather: tile t, window row q -> sorted rank r0 - 2*BAND + 116t + q
            rk = r0 - 2 * BAND + TR * tt + qq128                # [NTA, 128]
            ok = (rk >= 0) & (rk < N) & (qq128 < WIN)
            nodes = np.where(ok, order[np.clip(rk, 0, N - 1)], 0)

            # band1 [128 q, NTA t, BCOL r]: A1 row j = 116t + r is rank
            # g1 = r0 - 4 + j; value w9[g1, q - r] for 0 <= q-r <= 8.
            # Columns [116, 124) replicate the next tile's first rows; agg2
            # reads the +4-shifted window (band2[q,u,s] == band1[q+4,u,s+4]).
            g1 = r0 - BAND + TR * tt + rr                       # [NTA, BCOL]
            v1 = (g1 >= 0) & (g1 < N) & (g1 < r0 + HALF + BAND)
            band1 = w9g[np.clip(g1, 0, N - 1)[None], dd_c[:, None, :]]
            band1 *= (v1[None] & okd[:, None, :])

            per_core.append(dict(
                nodes=nodes, ok=ok, order=order,
                band1=np.ascontiguousarray(band1).astype(bf16),
            ))
    return per_core


# ---------------------------------------------------------------- device IR
def build_nc():
    import concourse.bacc as bacc
    import concourse.mybir as mybir
    from concourse.tile import TileContext

    F32 = mybir.dt.float32
    BF16 = mybir.dt.bfloat16
    RELU = mybir.ActivationFunctionType.Relu
    ADD = mybir.AluOpType.add

    nc = bacc.Bacc()
    xs = nc.dram_tensor("xs", [128, NTA, CIN], BF16, kind="ExternalInput")
    band1 = nc.dram_tensor("band1", [128, NTA, BCOL], BF16, kind="ExternalInput")
    w1 = nc.dram_tensor("w1", [128, 2, CHID], BF16, kind="ExternalInput")
    w2 = nc.dram_tensor("w2", [128, 4, COUT], BF16, kind="ExternalInput")
    b1 = nc.dram_tensor("b1", [128, 4], F32, kind="ExternalInput")
    b2rep = nc.dram_tensor("b2rep", [TR, COUT], BF16, kind="ExternalInput")
    out_s = nc.dram_tensor("out_s", [TR, NTA, COUT], BF16, kind="ExternalOutput")

    with TileContext(nc) as tc:
        with (
            tc.tile_pool(name="const", bufs=1) as cpool,
            tc.tile_pool(name="big", bufs=1) as big,
            tc.tile_pool(name="psum", bufs=2, space="PSUM") as pp,
        ):
            # interleave xs (sync HWDGE ring) and band1 (scalar HWDGE ring)
            # per 3-tile group so agg1 can start on tile 0 ASAP
            xs_sb = cpool.tile([128, NTA, CIN], BF16)
            band1_sb = cpool.tile([128, NTA, BCOL], BF16)
            # Each HWDGE ring executes its DMAs serially in FIFO order, so
            # ring assignment = arrival schedule. sync: xs chunks (the agg1
            # critical path). scalar: w1 then band1 chunks in tile order.
            # gpsimd (SWDGE): everything needed later.
            w1_sb = cpool.tile([128, 2, CHID], BF16)
            b1_sb = cpool.tile([128, 4], F32)
            w2_sb = cpool.tile([128, 4, COUT], BF16)
            b2_sb = cpool.tile([TR, COUT], BF16)
            # Few BIG DMAs: small DMAs on a serial HWDGE ring only reach
            # ~100-150 GB/s; ~0.5MB+ transfers run near peak.
            for lo_t, hi_t in ((0, 3), (3, 6), (6, 12), (12, 18)):
                nc.sync.dma_start(xs_sb[:, lo_t:hi_t, :], xs[:, lo_t:hi_t, :])
            nc.scalar.dma_start(band1_sb[:, 0:6, :], band1[:, 0:6, :])
            nc.scalar.dma_start(w1_sb, w1[:, :, :])
            nc.scalar.dma_start(band1_sb[:, 6:NTA, :], band1[:, 6:NTA, :])
            nc.gpsimd.dma_start(b1_sb, b1[:, :])
            nc.gpsimd.dma_start(w2_sb, w2[:, :, :])
            nc.gpsimd.dma_start(b2_sb, b2rep[:, :])

            a1T = big.tile([128, 2, NCOLP], BF16)
            # hT columns are shifted by +4: col 4+j holds a1 col j, cols
            # [0,4) are zero. dense2 tile u then reads cols [116u, 116u+128)
            # giving a 128-row T2 tile whose row p sits at rank r0-8+116u+p,
            # so agg2's band slice is band1[:, u, 4:120] with base 0.
            hT = big.tile([128, 4, NCOLP + 4], BF16)
            t2n = big.tile([128, NTA, COUT], BF16)
            out_sb = big.tile([TR, NTA, COUT], BF16)

            # PE prewarm: ~3.4us of dummy matmuls (one full HAM window) so
            # the clock-gate opens to 2.4 GHz right as the first input chunk
            # lands; they only depend on the DVE memset, so they run under
            # the input-DMA wait.
            warm_sb = cpool.tile([128, 512], BF16)
            nc.vector.memset(warm_sb, 0.25)
            for _ in range(8):
                psW = pp.tile([128, 512], F32, tag="d1", bufs=3)
                nc.tensor.matmul(psW, lhsT=warm_sb[:, 0:128], rhs=warm_sb,
                                 start=True, stop=True)

            # zero the padded tail/head so dense1/dense2 read finite values
            # (on DVE: the gpsimd queue is busy generating SWDGE descriptors)
            for cb in range(2):
                nc.vector.memset(a1T[:, cb, NCOL:NCOLP], 0.0)
            for mb in range(4):
                nc.vector.memset(hT[:, mb, 0:4], 0.0)

            MAX = mybir.AluOpType.max
            COPY = mybir.ActivationFunctionType.Copy

            def emit_agg1(t):
                # A1^T tile = xs_tile^T @ band1_tile (K = 124-row window).
                # Both cin-halves land in one 2-region psum tile -> a single
                # drain, so the PE issues back-to-back (keeps HAM warm).
                psA = pp.tile([128, 2, TR], F32, tag="sm", bufs=3)
                for cb in range(2):
                    nc.tensor.matmul(psA[:, cb, :],
                                     lhsT=xs_sb[0:WIN, t, 128 * cb:128 * (cb + 1)],
                                     rhs=band1_sb[0:WIN, t, 0:TR],
                                     start=True, stop=True)
                dst = a1T[:, :, TR * t:TR * (t + 1)]
                if t % 2 == 0:
                    nc.vector.tensor_copy(dst, psA)
                else:
                    nc.scalar.activation(dst, psA, COPY)

            def emit_d1(lo, hi, mb):
                # H^T block = relu(W1^T @ A1^T + b1)
                psH = pp.tile([128, 512], F32, tag="d1", bufs=3)
                for kb in range(2):
                    nc.tensor.matmul(psH[:, 0:hi - lo],
                                     lhsT=w1_sb[:, kb, 128 * mb:128 * (mb + 1)],
                                     rhs=a1T[:, kb, lo:hi],
                                     start=(kb == 0), stop=(kb == 1))
                if mb % 2 == 0:
                    nc.scalar.activation(hT[:, mb, lo + 4:hi + 4],
                                         psH[:, 0:hi - lo], RELU,
                                         bias=b1_sb[:, mb:mb + 1], scale=1.0)
                else:
                    nc.vector.tensor_scalar(
                        out=hT[:, mb, lo + 4:hi + 4], in0=psH[:, 0:hi - lo],
                        scalar1=b1_sb[:, mb:mb + 1], scalar2=0.0,
                        op0=ADD, op1=MAX)

            def emit_d2(us):
                # one psum tile + one drain per GROUP of consecutive tiles
                n = len(us)
                psT = pp.tile([128, 2, COUT], F32, tag="d2")
                for i, u in enumerate(us):
                    for kb in range(4):
                        nc.tensor.matmul(psT[:, i, :],
                                         lhsT=hT[:, kb, TR * u:TR * u + 128],
                                         rhs=w2_sb[:, kb, :],
                                         start=(kb == 0), stop=(kb == 3))
                dst = t2n[:, us[0]:us[0] + n, :]
                if us[0] % 2 == 0:
                    nc.scalar.activation(dst, psT[:, 0:n, :], COPY)
                else:
                    nc.vector.tensor_copy(dst, psT[:, 0:n, :])

            def emit_a2(u, flush=None):
                # band2[q,u,s] == band1[q+4,u,s+4]; with hT's +4 column
                # shift, t2n row p sits at rank r0-8+116u+p, so the band
                # slice is a pure free-dim shift with partition base 0.
                psO = pp.tile([TR, COUT], F32, tag="sm", bufs=3)
                nc.tensor.matmul(psO,
                                 lhsT=band1_sb[:, u, 4:TR + 4],
                                 rhs=t2n[:, u, :], start=True, stop=True)
                nc.vector.tensor_tensor(out=out_sb[:, u, :], in0=psO,
                                        in1=b2_sb, op=ADD)
                nc.scalar.activation(out_sb[:, u, :], out_sb[:, u, :], RELU)
                if flush is not None:
                    lo, hi = flush
                    nc.sync.dma_start(out_s[:, lo:hi, :],
                                      out_sb[:, lo:hi, :])

            # ---------------- fused pipeline, chasing the input DMA stream:
            # per 6-tile chunk: agg1 -> dense1 (348-col blocks) -> all dense2/
            # agg2 tiles whose inputs just completed. PE work per chunk
            # (~6.5us) far exceeds the chunk arrival cadence (~2us), so the
            # PE never stalls (or re-throttles) once chunk 0 lands.
            # dense2/agg2 in groups; tile 17 (the 8-valid-row partial tile)
            # is finished early in the last chunk so the final flush is tiny.
            d2_groups = [[(0, 1), (2, 3), (4,)],
                         [(5, 6), (7, 8), (9, 10)],
                         [(11, 12), (13,), (17,), (14, 15), (16,)]]
            flushes = {2: (0, 3), 5: (3, 6), 8: (6, 9), 11: (9, 12),
                       13: (12, 14), 17: (17, 18), 15: (14, 16), 16: (16, 17)}
            CW = 6 * TR   # a1/h columns per 6-tile chunk

            def d1_blocks(ch):
                for half_blk in range(2):
                    lo = CW * ch + 348 * half_blk
                    hi = lo + 348
                    if ch == 2 and half_blk == 1:
                        hi = NCOLP
                    for mb in range(4):
                        yield (lo, hi, mb)

            # agg1 of chunk ch+1 is interleaved with dense1 of chunk ch so
            # the PE always has an independent matmul to issue while drains
            # complete (keeps the issue stream dense -> HAM stays warm).
            pend = None   # d2 group whose agg2 is still outstanding
            for t in range(6):
                emit_agg1(t)
            for ch in range(3):
                next_tiles = list(range(6 * ch + 6, min(6 * ch + 12, NTA)))
                for i, (lo, hi, mb) in enumerate(d1_blocks(ch)):
                    emit_d1(lo, hi, mb)
                    if i < len(next_tiles):
                        emit_agg1(next_tiles[i])
                for grp in d2_groups[ch]:
                    emit_d2(list(grp))
                    if pend is not None:
                        for u in pend:
                            emit_a2(u, flushes.get(u))
                    pend = grp
            for u in pend:
                emit_a2(u, flushes.get(u))

    nc.compile()
    return nc


# ---------------------------------------------------------------- host glue
def make_in_maps(density_maps, feature_maps, W1, b1, W2, b2):
    import ml_dtypes
    bf16 = ml_dtypes.bfloat16

    graph = _host_graph(density_maps)
    fm = np.asarray(feature_maps, dtype=np.float32)
    fmT = [np.ascontiguousarray(fm[b].reshape(CIN, N).T) for b in range(B)]

    w1d = np.ascontiguousarray(
        np.asarray(W1, np.float32).reshape(2, 128, CHID).transpose(1, 0, 2)
    ).astype(bf16)
    w2d = np.ascontiguousarray(
        np.asarray(W2, np.float32).reshape(4, 128, COUT).transpose(1, 0, 2)
    ).astype(bf16)
    b1d = np.ascontiguousarray(np.asarray(b1, np.float32).reshape(4, 128).T)
    b2d = np.ascontiguousarray(
        np.broadcast_to(np.asarray(b2, np.float32), (TR, COUT))).astype(bf16)

    in_maps = []
    for c in range(8):
        g = graph[c]
        xs = fmT[c // 2][g["nodes"].reshape(-1)].reshape(NTA, 128, CIN)
        xs[~g["ok"]] = 0.0
        xs_dev = xs.transpose(1, 0, 2).astype(bf16)
        in_maps.append({
            "xs": xs_dev, "band1": g["band1"],
            "w1": w1d, "w2": w2d, "b1": b1d, "b2rep": b2d,
        })
    return in_maps, graph


def kernel(density_maps, feature_maps, W1, b1, W2, b2):
    from concourse.bass_utils import run_bass_kernel_spmd

    if "nc" not in _COMPILED:
        _COMPILED["nc"] = build_nc()
    nc = _COMPILED["nc"]

    in_maps, graph = make_in_maps(density_maps, feature_maps, W1, b1, W2, b2)
    res = run_bass_kernel_spmd(nc, in_maps, core_ids=list(range(8)))

    out = np.empty((B, N, COUT), dtype=np.float32)
    for b in range(B):
        order = graph[2 * b]["order"]
        for half in range(2):
            o = np.asarray(res.results[2 * b + half]["out_s"]).astype(np.float32)
            vals = o.transpose(1, 0, 2).reshape(NCOL, COUT)[:HALF]
            out[b][order[half * HALF:(half + 1) * HALF]] = vals
    return np.ascontiguousarray(
        out.reshape(B, H, W, COUT).transpose(0, 3, 1, 2)).astype(np.float32)


# revision 40
# speedup vs baseline: 1.0641x; 1.0011x over previous
"""Trainium2 Bass kernel for DensityGCNProcessor.

Model: 2-layer GCN over a per-sample kNN graph built from 1-D density values
(K=4 nearest by |density_i - density_j|), symmetric deg^-1/2 normalization on
target indegree, relu after each layer.

Strategy
--------
kNN in a 1-D metric means: after sorting nodes by density, every node's 4
nearest neighbours lie within +/-4 sorted positions, so each aggregation is a
9-diagonal banded matrix in sorted order. The host does all O(N log N) index
math (argsort, band weights with exact reference tie-breaking) and also the
data-dependent *permutation* of the inputs/outputs (gather into sorted order
is part of sharding). The device kernel is then a pure dense pipeline with
zero transposes and zero indirect DMA:

  agg1 (flip):  A1^T[cin,rows] = xs_tile^T @ band1_tile     (TensorE)
  dense1:       H^T[chid,cols] = relu(W1^T @ A1^T + b1)     (TensorE + ACT)
  dense2:       T2[rows,cout]  = (H^T tile)^T @ W2          (TensorE)
  agg2:         out[rows,cout] = relu(band2^T @ T2 + b2)    (TensorE + DVE/ACT)

Node tiles are 120 rows wide so each tile's +/-4-halo window is exactly 128
sorted rows -> every band aggregation is a single K=128 matmul. All operands
are bf16 (tolerance is 2e-2; measured headroom ~30x), all DMAs are contiguous
per partition. A ~3.4us prewarm matmul burst (dep: one DVE memset) runs under
the input-DMA wait so the HAM clock gate opens to 2.4 GHz before the real
burst starts.

Sharding: 8 cores = 4 batches x 2 rank-halves. Core c handles batch c//2,
sorted ranks [ (c%2)*2048, (c%2)*2048+2048 ).
"""

import numpy as np

# ---------------------------------------------------------------- constants
B = 4
CIN = 256
CHID = 512
COUT = 256
H = W = 64
N = H * W            # 4096 nodes per batch
KNN = 4
BAND = 4             # kNN lies within +/-4 sorted positions
HALF = N // 2        # 2048 ranks per core
TR = 120             # rows per node tile (window = TR + 2*BAND = 128)
NTA = 18             # node tiles (covers 2160 rows >= 2048 + halo)
NCOL = NTA * TR      # 2160
NCOLP = NCOL + 16    # padded a1T/hT column space (dense2 tile 17 reads 2168)

_COMPILED = {}


# ---------------------------------------------------------------- host graph
def _build_band_weights(d_flat):
    """order [N], w9 [N, 9] f32: out_s[r] = sum_o w9[r, o+4] * g_s[r+o]."""
    order = np.argsort(d_flat, kind="stable")
    d_s = d_flat[order]

    offs = np.arange(-BAND, BAND + 1)
    ridx = np.arange(N)[:, None] + offs[None, :]
    valid = (ridx >= 0) & (ridx < N)
    ridx_c = np.clip(ridx, 0, N - 1)
    c = np.abs(d_s[ridx_c] - d_s[:, None]).astype(np.float32)
    c = np.where(valid, c, np.float32(np.inf))
    cand_j = np.where(valid, order[ridx_c], N)

    # reference = stable argsort over the full row: ties by smaller orig index.
    sel = np.lexsort((cand_j, c), axis=1)
    tgt_s = np.take_along_axis(ridx_c, sel[:, 1:KNN + 1], axis=1).reshape(-1)
    src_s = np.repeat(np.arange(N), KNN)

    deg = np.ones(N, dtype=np.float32)
    np.add.at(deg, tgt_s, np.float32(1.0))
    dinv = (np.float32(1.0) / np.sqrt(deg)).astype(np.float32)

    m = np.zeros((N, 9), dtype=np.float32)
    np.add.at(m, (tgt_s, src_s - tgt_s + BAND), np.float32(1.0))
    m[:, BAND] += 1.0  # self loops

    ro = np.arange(N)[:, None] + offs[None, :]
    rov = (ro >= 0) & (ro < N)
    w9 = m * dinv[:, None] * dinv[np.clip(ro, 0, N - 1)] * rov
    return order.astype(np.int64), w9.astype(np.float32)


def _host_graph(density_maps):
    """Per-core index/band tensors. Returns list of 8 dicts."""
    import ml_dtypes
    bf16 = ml_dtypes.bfloat16

    tt = np.arange(NTA)[:, None]
    qq128 = np.arange(128)[None, :]
    rr = np.arange(TR)[None, :]
    dd = np.arange(128)[:, None] - np.arange(TR)[None, :]      # [128, TR]
    okd = (dd >= 0) & (dd <= 2 * BAND)
    dd_c = np.clip(dd, 0, 2 * BAND)

    per_core = []
    for b in range(B):
        d = np.asarray(density_maps[b]).reshape(N).astype(np.float32)
        order, w9g = _build_band_weights(d)
        for half in range(2):
            r0 = half * HALF

            # xs gather: tile t, window row q -> sorted rank r0 - 2*BAND + 120t + q
            rk = r0 - 2 * BAND + TR * tt + qq128                # [NTA, 128]
            ok = (rk >= 0) & (rk < N)
            nodes = np.where(ok, order[np.clip(rk, 0, N - 1)], 0)

            # band1 [128 q, NTA t, TR r]: A1 row j = 120t + r is rank
            # g1 = r0 - 4 + j; value w9[g1, q - r] for 0 <= q-r <= 8.
            g1 = r0 - BAND + TR * tt + rr                       # [NTA, TR]
            v1 = (g1 >= 0) & (g1 < N) & (g1 < r0 + HALF + BAND)
            band1 = w9g[np.clip(g1, 0, N - 1)[None], dd_c[:, None, :]]
            band1 *= (v1[None] & okd[:, None, :])

            # band2 [128 q, NTA u, TR s]: out row rank g2 = r0 + 120u + s,
            # window = T2 tile u rows q = s + o + 4, value w9[g2, q - s].
            g2 = r0 + TR * tt + rr
            v2 = (g2 < r0 + HALF) & (g2 < N)
            band2 = w9g[np.clip(g2, 0, N - 1)[None], dd_c[:, None, :]]
            band2 *= (v2[None] & okd[:, None, :])

            per_core.append(dict(
                nodes=nodes, ok=ok, order=order,
                band1=np.ascontiguousarray(band1).astype(bf16),
                band2=np.ascontiguousarray(band2).astype(bf16),
            ))
    return per_core


# ---------------------------------------------------------------- device IR
def build_nc():
    import concourse.bacc as bacc
    import concourse.mybir as mybir
    from concourse.tile import TileContext

    F32 = mybir.dt.float32
    BF16 = mybir.dt.bfloat16
    RELU = mybir.ActivationFunctionType.Relu
    ADD = mybir.AluOpType.add

    nc = bacc.Bacc()
    xs = nc.dram_tensor("xs", [128, NTA, CIN], BF16, kind="ExternalInput")
    band1 = nc.dram_tensor("band1", [128, NTA, TR], BF16, kind="ExternalInput")
    band2 = nc.dram_tensor("band2", [128, NTA, TR], BF16, kind="ExternalInput")
    w1 = nc.dram_tensor("w1", [128, 2, CHID], BF16, kind="ExternalInput")
    w2 = nc.dram_tensor("w2", [128, 4, COUT], BF16, kind="ExternalInput")
    b1 = nc.dram_tensor("b1", [128, 4], F32, kind="ExternalInput")
    b2rep = nc.dram_tensor("b2rep", [TR, COUT], F32, kind="ExternalInput")
    out_s = nc.dram_tensor("out_s", [TR, NTA, COUT], F32, kind="ExternalOutput")

    with TileContext(nc) as tc:
        with (
            tc.tile_pool(name="const", bufs=1) as cpool,
            tc.tile_pool(name="big", bufs=1) as big,
            tc.tile_pool(name="psum", bufs=2, space="PSUM") as pp,
        ):
            # PE prewarm: ~3.4us of dummy matmuls (one full HAM window),
            # gated only on a DVE memset, so they run under the input-DMA
            # wait and the clock gate is open when the real burst starts.
            warm_sb = cpool.tile([128, 512], BF16)
            nc.vector.memset(warm_sb, 0.25)
            for _ in range(8):
                psW = pp.tile([128, 512], F32, tag="d1")
                nc.tensor.matmul(psW, lhsT=warm_sb[:, 0:128], rhs=warm_sb,
                                 start=True, stop=True)

            w1_sb = cpool.tile([128, 2, CHID], BF16)
            nc.scalar.dma_start(w1_sb, w1[:, :, :])
            w2_sb = cpool.tile([128, 4, COUT], BF16)
            nc.scalar.dma_start(w2_sb, w2[:, :, :])
            b1_sb = cpool.tile([128, 4], F32)
            nc.scalar.dma_start(b1_sb, b1[:, :])
            b2_sb = cpool.tile([TR, COUT], F32)
            nc.scalar.dma_start(b2_sb, b2rep[:, :])
            band1_sb = cpool.tile([128, NTA, TR], BF16)
            nc.gpsimd.dma_start(band1_sb, band1[:, :, :])
            band2_sb = cpool.tile([128, NTA, TR], BF16)
            nc.gpsimd.dma_start(band2_sb, band2[:, :, :])

            xs_sb = cpool.tile([128, NTA, CIN], BF16)
            for ch in range(6):
                nc.sync.dma_start(xs_sb[:, 3 * ch:3 * (ch + 1), :],
                                  xs[:, 3 * ch:3 * (ch + 1), :])

            a1T = big.tile([128, 2, NCOLP], BF16)
            hT = big.tile([128, 4, NCOLP], BF16)
            t2n = big.tile([128, NTA, COUT], BF16)
            out_sb = big.tile([TR, NTA, COUT], F32)

            # zero the padded tail so dense1/dense2 read finite values there
            for cb in range(2):
                nc.vector.memset(a1T[:, cb, NCOL:NCOLP], 0.0)

            # ---------------- agg1: A1^T tiles = xs_tile^T @ band1_tile
            for t in range(NTA):
                for cb in range(2):
                    psA = pp.tile([128, TR], F32, tag="agg1")
                    nc.tensor.matmul(psA,
                                     lhsT=xs_sb[:, t, 128 * cb:128 * (cb + 1)],
                                     rhs=band1_sb[:, t, :],
                                     start=True, stop=True)
                    dst = a1T[:, cb, TR * t:TR * (t + 1)]
                    if cb == 0:
                        nc.vector.tensor_copy(dst, psA)
                    else:
                        nc.scalar.activation(dst, psA,
                                             mybir.ActivationFunctionType.Copy)

            # ---------------- dense1: H^T = relu(W1^T @ A1^T + b1)
            blocks = [(0, 512), (512, 1024), (1024, 1536), (1536, 2048),
                      (2048, NCOLP)]
            for lo, hi in blocks:
                for mb in range(4):
                    psH = pp.tile([128, 512], F32, tag="d1")
                    for kb in range(2):
                        nc.tensor.matmul(psH[:, 0:hi - lo],
                                         lhsT=w1_sb[:, kb, 128 * mb:128 * (mb + 1)],
                                         rhs=a1T[:, kb, lo:hi],
                                         start=(kb == 0), stop=(kb == 1))
                    nc.scalar.activation(hT[:, mb, lo:hi], psH[:, 0:hi - lo],
                                         RELU, bias=b1_sb[:, mb:mb + 1],
                                         scale=1.0)

            # ---------------- dense2 + agg2, interleaved per tile
            def emit_d2(u):
                psT = pp.tile([128, COUT], F32, tag="d2")
                for kb in range(4):
                    nc.tensor.matmul(psT,
                                     lhsT=hT[:, kb, TR * u:TR * u + 128],
                                     rhs=w2_sb[:, kb, :],
                                     start=(kb == 0), stop=(kb == 3))
                nc.vector.tensor_copy(t2n[:, u, :], psT)

            def emit_a2(u):
                psO = pp.tile([TR, COUT], F32, tag="a2")
                nc.tensor.matmul(psO, lhsT=band2_sb[:, u, :],
                                 rhs=t2n[:, u, :], start=True, stop=True)
                nc.vector.tensor_tensor(out=out_sb[:, u, :], in0=psO,
                                        in1=b2_sb, op=ADD)
                nc.scalar.activation(out_sb[:, u, :], out_sb[:, u, :], RELU)
                flush = {4: 0, 9: 5, 14: 10, NTA - 1: 15}
                if u in flush:
                    lo = flush[u]
                    nc.sync.dma_start(out_s[:, lo:u + 1, :],
                                      out_sb[:, lo:u + 1, :])

            emit_d2(0)
            for u in range(1, NTA):
                emit_d2(u)
                emit_a2(u - 1)
            emit_a2(NTA - 1)

    nc.compile()
    return nc


# ---------------------------------------------------------------- host glue
def make_in_maps(density_maps, feature_maps, W1, b1, W2, b2):
    import ml_dtypes
    bf16 = ml_dtypes.bfloat16

    graph = _host_graph(density_maps)
    fm = np.asarray(feature_maps, dtype=np.float32)
    fmT = [np.ascontiguousarray(fm[b].reshape(CIN, N).T) for b in range(B)]

    w1d = np.ascontiguousarray(
        np.asarray(W1, np.float32).reshape(2, 128, CHID).transpose(1, 0, 2)
    ).astype(bf16)
    w2d = np.ascontiguousarray(
        np.asarray(W2, np.float32).reshape(4, 128, COUT).transpose(1, 0, 2)
    ).astype(bf16)
    b1d = np.ascontiguousarray(np.asarray(b1, np.float32).reshape(4, 128).T)
    b2d = np.ascontiguousarray(
        np.broadcast_to(np.asarray(b2, np.float32), (TR, COUT)))

    in_maps = []
    for c in range(8):
        g = graph[c]
        xs = fmT[c // 2][g["nodes"].reshape(-1)].reshape(NTA, 128, CIN)
        xs[~g["ok"]] = 0.0
        xs_dev = xs.transpose(1, 0, 2).astype(bf16)
        in_maps.append({
            "xs": xs_dev, "band1": g["band1"], "band2": g["band2"],
            "w1": w1d, "w2": w2d, "b1": b1d, "b2rep": b2d,
        })
    return in_maps, graph


def kernel(density_maps, feature_maps, W1, b1, W2, b2):
    from concourse.bass_utils import run_bass_kernel_spmd

    if "nc" not in _COMPILED:
        _COMPILED["nc"] = build_nc()
    nc = _COMPILED["nc"]

    in_maps, graph = make_in_maps(density_maps, feature_maps, W1, b1, W2, b2)
    res = run_bass_kernel_spmd(nc, in_maps, core_ids=list(range(8)))

    out = np.empty((B, N, COUT), dtype=np.float32)
    for b in range(B):
        order = graph[2 * b]["order"]
        for half in range(2):
            o = np.asarray(res.results[2 * b + half]["out_s"], np.float32)
            vals = o.transpose(1, 0, 2).reshape(NCOL, COUT)[:HALF]
            out[b][order[half * HALF:(half + 1) * HALF]] = vals
    return np.ascontiguousarray(
        out.reshape(B, H, W, COUT).transpose(0, 3, 1, 2)).astype(np.float32)
